# revision 3
# baseline (speedup 1.0000x reference)
"""AFT-Full attention on 8 TRN2 NeuronCores (Bass/Tile, no collectives).

Reference math (B=2, TQ=TKV=512, DIM=512, HID=128, BDIM=128):
    qh  = q @ qW_w.T + qW_b
    k   = kv @ kW_w.T + kW_b
    v   = kv @ vW_w.T + vW_b
    wb  = w_bias_u @ w_bias_v                       # (TQ, TKV)
    A   = exp(k[:,None] + wb[None,:,:,None])        # (B,TQ,TKV,HID)
    out = sigmoid(qh) * (sum_s A*v / sum_s A)

Factorization used here: exp(k + wb) = exp(k) * exp(wb), so with
E = exp(k - kW_b·0 ...):
    num[t,h] = sum_s exp(wb[t,s]) * (exp(k0[s,h]) * v0[s,h])
    den[t,h] = sum_s exp(wb[t,s]) *  exp(k0[s,h])
where k0/v0 are the bias-free projections.  The k-bias cancels exactly in
num/den; the v-bias is a pure per-h additive term:
    out = sigmoid(qh) * (num0/den0 + vW_b)

Sharding: the 1024 flattened (b, t) query rows are split into 8 blocks of
128 — core i handles batch b=i//4, queries t in [128*(i%4), 128*(i%4+1)).
Each core only needs kv[b], so there is no cross-core communication at all
(collectives have a ~7us latency floor, larger than this whole kernel).

All host-side reshapes feed the device natural matmul layouts (contraction
dim on partitions), so the device performs zero transposes:
    ek/v   (s,h): lhsT = kvT (d,s) chunks,  rhs = [kW|vW]T (d,2h)
    expwbT (s,t): lhsT = w_bias_v (c,s),    rhs = uT (c,t)
    qhT    (h,t): lhsT = qWT (d,h),         rhs = qT (d,t)
    numT   (h,t): lhsT = ekv (s,h),         rhs = expwbT (s,t)
    denT   (h,t): lhsT = ek  (s,h),         rhs = expwbT (s,t)
Matmuls run in bf16 (f32 PSUM accumulate); DMA stays f32.
"""

import numpy as np

import concourse.bass as bass
import concourse.mybir as mybir
import concourse.tile as tile
from concourse import bacc
from concourse.bass_utils import run_bass_kernel_spmd

B, TQ, TKV, DIM, HID, BDIM = 2, 512, 512, 512, 128, 128
N_CORES = 8
R = (B * TQ) // N_CORES  # 128 query rows per core
P = 128
DC = DIM // P  # 4 contraction chunks for d
SC = TKV // P  # 4 contraction chunks for s
F32 = mybir.dt.float32
BF16 = mybir.dt.bfloat16
ACT = mybir.ActivationFunctionType


def _build():
    nc = bacc.Bacc(None)
    kvT = nc.declare_dram_parameter("kvT", [DIM, TKV], F32, isOutput=False)
    kvWT = nc.declare_dram_parameter("kvWT", [DIM, 2 * HID], F32, isOutput=False)
    qT = nc.declare_dram_parameter("qT", [DIM, R], F32, isOutput=False)
    qWT = nc.declare_dram_parameter("qWT", [DIM, HID], F32, isOutput=False)
    wbv = nc.declare_dram_parameter("wbv", [BDIM, TKV], F32, isOutput=False)
    uT = nc.declare_dram_parameter("uT", [BDIM, R], F32, isOutput=False)
    qb = nc.declare_dram_parameter("qb", [HID, 1], F32, isOutput=False)
    vb = nc.declare_dram_parameter("vb", [HID, 1], F32, isOutput=False)
    out = nc.declare_dram_parameter("out", [HID, R], F32, isOutput=True)

    with tile.TileContext(nc) as tc:
        with (
            tc.tile_pool(name="stage", bufs=2) as stage,
            tc.tile_pool(name="persist", bufs=1) as persist,
            tc.tile_pool(name="psum", bufs=2, space="PSUM") as psum,
            tc.tile_pool(name="psum1", bufs=1, space="PSUM") as psum1,
        ):
            # ---- DMA loads (HWDGE/SP ring, FIFO: critical-path first) ----
            kvWT_st = persist.tile([P, DC, 2 * HID], F32, tag="kvWT_st")
            nc.sync.dma_start(
                out=kvWT_st[:], in_=kvWT[:].rearrange("(c p) n -> p c n", p=P)
            )
            kv_st = persist.tile([P, DC, TKV], F32, tag="kv_st")
            for dc in range(DC):
                nc.sync.dma_start(
                    out=kv_st[:, dc, :], in_=kvT[dc * P : (dc + 1) * P, :]
                )
            wbv_st = persist.tile([P, TKV], F32, tag="wbv_st")
            nc.sync.dma_start(out=wbv_st[:], in_=wbv[:])
            uT_st = persist.tile([P, R], F32, tag="uT_st")
            nc.sync.dma_start(out=uT_st[:], in_=uT[:])
            qWT_st = persist.tile([P, DC, HID], F32, tag="qWT_st")
            nc.sync.dma_start(
                out=qWT_st[:], in_=qWT[:].rearrange("(c p) n -> p c n", p=P)
            )
            qT_st = persist.tile([P, DC, R], F32, tag="qT_st")
            nc.sync.dma_start(
                out=qT_st[:], in_=qT[:].rearrange("(c p) n -> p c n", p=P)
            )
            qb_sb = persist.tile([P, 1], F32, tag="qb_sb")
            nc.sync.dma_start(out=qb_sb[:], in_=qb[:])
            vb_sb = persist.tile([P, 1], F32, tag="vb_sb")
            nc.sync.dma_start(out=vb_sb[:], in_=vb[:])

            # ---- casts to bf16 (DVE) ----
            kvW_bf = persist.tile([P, DC, 2 * HID], BF16, tag="kvW_bf")
            nc.vector.tensor_copy(kvW_bf[:], kvWT_st[:])
            kv_bf = persist.tile([P, DC, TKV], BF16, tag="kv_bf")
            for dc in range(DC):
                nc.vector.tensor_copy(kv_bf[:, dc, :], kv_st[:, dc, :])
            wbv_bf = persist.tile([P, TKV], BF16, tag="wbv_bf")
            nc.vector.tensor_copy(wbv_bf[:], wbv_st[:])
            uT_bf = persist.tile([P, R], BF16, tag="uT_bf")
            nc.vector.tensor_copy(uT_bf[:], uT_st[:])
            qWT_bf = persist.tile([P, DC, HID], BF16, tag="qWT_bf")
            nc.vector.tensor_copy(qWT_bf[:], qWT_st[:])
            qT_bf = persist.tile([P, DC, R], BF16, tag="qT_bf")
            nc.vector.tensor_copy(qT_bf[:], qT_st[:])

            # ---- expwbT (s,t) = exp(w_bias_v.T @ w_bias_u.T slice) ----
            wT_bf = persist.tile([P, SC, R], BF16, tag="wT_bf")
            for sc in range(SC):
                pw = psum.tile([P, R], F32, tag="pw")
                nc.tensor.matmul(
                    pw[:], lhsT=wbv_bf[:, sc * P : (sc + 1) * P], rhs=uT_bf[:]
                )
                nc.scalar.activation(wT_bf[:, sc, :], pw[:], ACT.Exp)

            # ---- k/v projections -> ek=exp(k0), ekv=ek*v0  (s,h) ----
            ek_bf = persist.tile([P, SC, HID], BF16, tag="ek_bf")
            ekv_bf = persist.tile([P, SC, HID], BF16, tag="ekv_bf")
            for sc in range(SC):
                pkv = psum.tile([P, 2 * HID], F32, tag="pkv")
                for dc in range(DC):
                    nc.tensor.matmul(
                        pkv[:],
                        lhsT=kv_bf[:, dc, sc * P : (sc + 1) * P],
                        rhs=kvW_bf[:, dc, :],
                        start=(dc == 0),
                        stop=(dc == DC - 1),
                    )
                nc.scalar.activation(ek_bf[:, sc, :], pkv[:, :HID], ACT.Exp)
                v_bf = stage.tile([P, HID], BF16, tag="v_bf")
                nc.vector.tensor_copy(v_bf[:], pkv[:, HID:])
                nc.vector.tensor_mul(ekv_bf[:, sc, :], ek_bf[:, sc, :], v_bf[:])

            # ---- qhT (h,t) -> sigmoid(qh + qW_b) ----
            pq = psum1.tile([P, R], F32, tag="pq")
            for dc in range(DC):
                nc.tensor.matmul(
                    pq[:],
                    lhsT=qWT_bf[:, dc, :],
                    rhs=qT_bf[:, dc, :],
                    start=(dc == 0),
                    stop=(dc == DC - 1),
                )
            sig_sb = persist.tile([P, R], F32, tag="sig_sb")
            nc.scalar.activation(sig_sb[:], pq[:], ACT.Sigmoid, bias=qb_sb[:])

            # ---- numT/denT (h,t) ----
            pn = psum1.tile([P, R], F32, tag="pn")
            for sc in range(SC):
                nc.tensor.matmul(
                    pn[:],
                    lhsT=ekv_bf[:, sc, :],
                    rhs=wT_bf[:, sc, :],
                    start=(sc == 0),
                    stop=(sc == SC - 1),
                )
            pd = psum1.tile([P, R], F32, tag="pd")
            for sc in range(SC):
                nc.tensor.matmul(
                    pd[:],
                    lhsT=ek_bf[:, sc, :],
                    rhs=wT_bf[:, sc, :],
                    start=(sc == 0),
                    stop=(sc == SC - 1),
                )

            # ---- out = sigmoid(qhT) * (numT/denT + vW_b) ----
            rec_sb = persist.tile([P, R], F32, tag="rec_sb")
            nc.vector.reciprocal(rec_sb[:], pd[:])
            res_sb = persist.tile([P, R], F32, tag="res_sb")
            nc.vector.tensor_mul(res_sb[:], pn[:], rec_sb[:])
            nc.vector.tensor_scalar_add(res_sb[:], res_sb[:], vb_sb[:])
            nc.vector.tensor_mul(res_sb[:], res_sb[:], sig_sb[:])
            nc.sync.dma_start(out=out[:], in_=res_sb[:])

    nc.finalize()
    return nc


_NC_CACHE = None


def _get_nc():
    global _NC_CACHE
    if _NC_CACHE is None:
        _NC_CACHE = _build()
    return _NC_CACHE


def _make_in_maps(q, kv, qW_w, qW_b, kW_w, kW_b, vW_w, vW_b, w_bias_u, w_bias_v):
    f = lambda a: np.ascontiguousarray(np.asarray(a, dtype=np.float32))
    q, kv = f(q), f(kv)
    kvWT = f(np.concatenate([kW_w, vW_w], axis=0).T)  # (DIM, 2*HID)
    qWT = f(np.asarray(qW_w).T)  # (DIM, HID)
    wbv = f(w_bias_v)  # (BDIM, TKV)
    qb = f(qW_b).reshape(HID, 1)
    vb = f(vW_b).reshape(HID, 1)
    qf = q.reshape(B * TQ, DIM)
    u = f(w_bias_u)
    kvT_b = [f(kv[b].T) for b in range(B)]  # (DIM, TKV)
    in_maps = []
    for i in range(N_CORES):
        b = i // (N_CORES // B)
        t0 = (i % (N_CORES // B)) * R
        in_maps.append(
            {
                "kvT": kvT_b[b],
                "kvWT": kvWT,
                "qT": f(qf[i * R : (i + 1) * R].T),
                "qWT": qWT,
                "wbv": wbv,
                "uT": f(u[t0 : t0 + R].T),
                "qb": qb,
                "vb": vb,
            }
        )
    return in_maps


def _run(in_maps, trace=False):
    nc = _get_nc()
    return run_bass_kernel_spmd(
        nc, in_maps, core_ids=list(range(N_CORES)), trace=trace
    )


def kernel(**inputs) -> np.ndarray:
    in_maps = _make_in_maps(**inputs)
    res = _run(in_maps)
    out = np.empty((B * TQ, HID), dtype=np.float32)
    for i in range(N_CORES):
        out[i * R : (i + 1) * R] = res.results[i]["out"].T
    return out.reshape(B, TQ, HID)
